# revision 18
# baseline (speedup 1.0000x reference)
"""Distributed Llama-attention Bass kernel for 8 TRN2 NeuronCores.

Sharding: tensor-parallel over heads (core c owns query heads 4c..4c+3 and
KV head c), per-head-pair AllGathers of attention outputs (bf16) pipelined
against later chunks, and a column-shard of wo so each core produces a
disjoint [2048, 512] column slice of the final output (no all-reduce).

v6: single-pass projections + scheduler-pinned gather loads --
  * per chunk, ONE streamed pass over hsT computes k, v and all 4 q heads
    (6 PSUM accumulators): hsT is read once total (16MB instead of 48MB)
    and each 1MB batch carries 12.6us of PE work, so the projection phase
    is compute-dense and the DMA engines never starve the PE;
  * wk/wv arrive host-packed as one [D, 256] tensor (512B DMA lines --
    the split tensors' 256B lines loaded at <80GB/s and stalled startup);
  * AllGathers split per head-pair (8 x 2MB) so the tail oproj can start
    after only half the last chunk's data has arrived;
  * oth gather DMAs are pinned behind a late anchor instruction with a
    no-sync scheduler edge: the Tile scheduler otherwise hoists them to
    right after their AG and their unsatisfied semaphore wait convoys the
    gpsimd queue (v5 lost ~40us per chunk boundary to this);
  * oproj matmuls run as ~2us quanta interleaved into the next-next
    attention's exp-wait bubbles; softmax epilogue uses
    reciprocal_approx_fast; tail: attention(3) -> oproj(2) covers
    AG(3a/3b) wire time -> oproj(3) in two head-pair phases.
"""

import math
import sys

import numpy as np

sys.path.insert(0, "/opt/trn_rl_repo")

import ml_dtypes  # noqa: E402

import concourse.bass as bass  # noqa: E402
import concourse.mybir as mybir  # noqa: E402
import concourse.tile as tile  # noqa: E402
from concourse import bacc  # noqa: E402
from concourse.bass_utils import run_bass_kernel_spmd  # noqa: E402
from concourse.masks import make_identity  # noqa: E402
from concourse.tile import add_dep_helper  # noqa: E402

F32 = mybir.dt.float32
BF16 = mybir.dt.bfloat16
Alu = mybir.AluOpType
Act = mybir.ActivationFunctionType

NCORES = 8
S = 2048
D = 4096
H = 32
HKV = 8
HD = 128
NH = H // NCORES          # 4 local query heads
QCOLS = NH * HD           # 512 local q-proj cols
CHUNK = 512               # s-chunk size
NCHUNK = S // CHUNK       # 4
DC = D // 128             # 32 d-chunks
SCALE = 1.0 / math.sqrt(HD)

_CACHED = {}


def _build_graph():
    nc = bacc.Bacc(
        "TRN2",
        target_bir_lowering=False,
        debug=False,
        num_devices=NCORES,
    )

    hsT_d = nc.dram_tensor("hsT", [D, S], BF16, kind="ExternalInput").ap()
    wq_d = nc.dram_tensor("wq", [D, QCOLS], BF16, kind="ExternalInput").ap()
    wkv_d = nc.dram_tensor("wkv", [D, 2 * HD], BF16, kind="ExternalInput").ap()
    wo_d = nc.dram_tensor("wo", [D, QCOLS], BF16, kind="ExternalInput").ap()
    cos_d = nc.dram_tensor("cos", [HD, S], BF16, kind="ExternalInput").ap()
    sin_d = nc.dram_tensor("sin", [HD, S], BF16, kind="ExternalInput").ap()
    out_d = nc.dram_tensor("out", [S, QCOLS], F32, kind="ExternalOutput").ap()

    hsT_v = hsT_d.rearrange("(i p) s -> p i s", p=128)
    wq_v = wq_d.rearrange("(i p) w -> p i w", p=128)
    wkv_v = wkv_d.rearrange("(i p) w -> p i w", p=128)
    wo_v = wo_d.rearrange("(i p) w -> p i w", p=128)

    with tile.TileContext(nc) as tc:
        with (
            tc.tile_pool(name="persist", bufs=1) as pp,
            tc.tile_pool(name="hsd", bufs=8) as hsdp,
            tc.tile_pool(name="qtp", bufs=2) as qtp,
            tc.tile_pool(name="otp", bufs=6) as otp,
            tc.tile_pool(name="ep", bufs=4) as ep,
            tc.tile_pool(name="rt", bufs=2) as rtp,
            # prj: k/v accumulators + interleaved-oproj psum ring;
            # sc: scores + rowsum/broadcast + v-transpose;
            # acc: q accumulators / AV pairs / tail-oproj accumulators
            tc.tile_pool(name="ps_prj", bufs=2, space="PSUM") as ps_prj,
            tc.tile_pool(name="ps_sc", bufs=2, space="PSUM") as ps_sc,
            tc.tile_pool(name="ps_acc", bufs=4, space="PSUM") as ps_acc,
            tc.tile_pool(name="dram", bufs=1, space="DRAM") as dram,
        ):
            # ---------------- persistent SBUF tensors ----------------
            wq_bf = pp.tile([128, DC, QCOLS], BF16, tag="wq")
            wkv_bf = pp.tile([128, DC, 2 * HD], BF16, tag="wkv")
            wo_bf = pp.tile([128, DC, QCOLS], BF16, tag="wo")
            cos_sb = pp.tile([HD, S], BF16, tag="cos")
            sin_sb = pp.tile([HD, S], BF16, tag="sin")
            kT_bf = pp.tile([HD, S], BF16, tag="kt")
            v_bf = pp.tile([128, S // 128, HD], BF16, tag="v")
            ident = pp.tile([128, 128], BF16, tag="id")
            ones_col = pp.tile([128, 1], BF16, tag="onc")
            ones_row = pp.tile([1, 128], BF16, tag="onr")

            # per-(chunk, head-pair) AllGather buffers: 2 local heads per
            # side -> 256 rows in, 2048 rows gathered
            ain = {}
            aall = {}
            for j in range(NCHUNK):
                for s in ("a", "b"):
                    ain[j, s] = dram.tile(
                        [2 * HD, CHUNK], BF16, tag=f"ain{j}{s}",
                        name=f"ain{j}{s}",
                    )
                    aall[j, s] = dram.tile(
                        [NCORES * 2 * HD, CHUNK], BF16, tag=f"aall{j}{s}",
                        addr_space="Shared", name=f"aall{j}{s}",
                    )

            # ---------------- constants ----------------
            make_identity(nc, ident[:])
            nc.gpsimd.memset(ones_col[:], 1.0)
            nc.gpsimd.memset(ones_row[:], 1.0)
            # cos/sin on the gpsimd queue so they don't delay weight/hs DMA
            nc.gpsimd.dma_start(out=cos_sb[:], in_=cos_d)
            nc.gpsimd.dma_start(out=sin_sb[:], in_=sin_d)

            # warm-up: the first weight DMA takes ~15us to land (cold
            # device) and an idle PE both wastes that window and drops the
            # HAM clock gate to 1.2GHz for the first projection chunk.
            # Dummy identity matmuls (output never read) keep the array
            # busy and warm until real work arrives.
            warm = ps_sc.tile([128, CHUNK], F32, tag="sc", name="warm")
            for _ in range(170):
                nc.tensor.matmul(
                    warm[:, 0:128], lhsT=ident[:], rhs=ident[:],
                    start=True, stop=True,
                )

            def load_weight(dst_bf, src_v, n=4):
                for i in range(0, DC, n):
                    nc.scalar.dma_start(
                        out=dst_bf[:, i : i + n, :], in_=src_v[:, i : i + n, :]
                    )

            def rope(psrc, dst_ap, sl):
                t1 = rtp.tile([128, CHUNK], BF16, tag="rt")
                t2 = rtp.tile([128, CHUNK], BF16, tag="rt")
                nc.vector.tensor_tensor(
                    out=t1[0:64, :], in0=psrc[64:128, :],
                    in1=sin_sb[0:64, sl], op=Alu.mult,
                )
                nc.vector.tensor_tensor(
                    out=t1[64:128, :], in0=psrc[0:64, :],
                    in1=sin_sb[64:128, sl], op=Alu.mult,
                )
                nc.vector.tensor_tensor(
                    out=t2[:], in0=psrc[:], in1=cos_sb[:, sl], op=Alu.mult
                )
                return nc.vector.tensor_tensor(
                    out=dst_ap, in0=t1[:], in1=t2[:], op=Alu.add
                )

            # -------- combined K/V/Q projection (one pass per chunk) -----
            def kvq_chunk(j, qT):
                s0 = j * CHUNK
                sl = bass.ds(s0, CHUNK)
                psk = ps_prj.tile([128, CHUNK], F32, tag="p")
                psv = ps_prj.tile([128, CHUNK], F32, tag="p")
                psq = [
                    ps_acc.tile([128, CHUNK], F32, tag="acc", name=f"q{h}")
                    for h in range(NH)
                ]
                NB = 2
                for ib in range(0, DC, NB):
                    hsd = hsdp.tile([128, NB, CHUNK], BF16, tag="hsd")
                    nc.sync.dma_start(
                        out=hsd[:], in_=hsT_v[:, ib : ib + NB, s0 : s0 + CHUNK]
                    )
                    for k in range(NB):
                        i = ib + k
                        st, sp = (i == 0), (i == DC - 1)
                        nc.tensor.matmul(
                            psk[:], lhsT=wkv_bf[:, i, 0:HD], rhs=hsd[:, k, :],
                            start=st, stop=sp,
                        )
                        nc.tensor.matmul(
                            psv[:], lhsT=wkv_bf[:, i, HD : 2 * HD],
                            rhs=hsd[:, k, :], start=st, stop=sp,
                        )
                        for h in range(NH):
                            nc.tensor.matmul(
                                psq[h][:],
                                lhsT=wq_bf[:, i, HD * h : HD * (h + 1)],
                                rhs=hsd[:, k, :], start=st, stop=sp,
                            )
                # drain order matters: the next attention's first score
                # needs qT[h0], then (diagonal tiles, late) kT; v last
                rope(psq[0][:], qT[:, 0, :], sl)
                rope(psq[1][:], qT[:, 1, :], sl)
                rope(psk[:], kT_bf[:, sl], sl)
                vT_sb = ep.tile([128, CHUNK], BF16, tag="vts", bufs=1)
                nc.vector.tensor_copy(out=vT_sb[:], in_=psv[:])
                rope(psq[2][:], qT[:, 2, :], sl)
                anchor = rope(psq[3][:], qT[:, 3, :], sl)
                # v computed as vT [hd, s]; PE-transpose back to [s, hd]
                psv2 = ps_sc.tile([128, 4, 128], BF16, tag="sc")
                for ss in range(CHUNK // 128):
                    nc.tensor.transpose(
                        psv2[:, ss, :],
                        vT_sb[:, 128 * ss : 128 * (ss + 1)],
                        ident[:],
                    )
                nc.vector.tensor_copy(
                    out=v_bf[:, 4 * j : 4 * (j + 1), :], in_=psv2[:]
                )
                return anchor

            # ---------------- softmax epilogue ----------------
            def emit_epilogue(j, side, pairs):
                last_dma = None
                for hloc, pso, racc in pairs:
                    # partition reduce 128 -> 1 with a single ones-matmul
                    psr = ps_sc.tile([1, CHUNK], F32, tag="sc")
                    nc.tensor.matmul(
                        psr[:], lhsT=ones_col[:], rhs=racc[:],
                        start=True, stop=True,
                    )
                    rc32 = ep.tile([1, CHUNK], F32, tag="rc32", bufs=1)
                    nc.vector.reciprocal_approx_fast(out=rc32[:], in_=psr[:])
                    rc = ep.tile([1, CHUNK], BF16, tag="rc", bufs=1)
                    nc.scalar.copy(out=rc[:], in_=rc32[:])
                    psb = ps_sc.tile([128, CHUNK], F32, tag="sc")
                    nc.tensor.matmul(
                        psb[:], lhsT=ones_row[:], rhs=rc[:],
                        start=True, stop=True,
                    )
                    ao = ep.tile([128, CHUNK], BF16, tag="ao", bufs=2)
                    nc.vector.tensor_tensor(
                        out=ao[:], in0=pso[:], in1=psb[:], op=Alu.mult
                    )
                    last_dma = nc.gpsimd.dma_start(
                        out=ain[j, side][HD * hloc : HD * (hloc + 1), :],
                        in_=ao[:],
                    )
                return last_dma

            def ag_trigger(j, side):
                nc.gpsimd.collective_compute(
                    "AllGather",
                    Alu.bypass,
                    replica_groups=[list(range(NCORES))],
                    ins=[ain[j, side].opt()],
                    outs=[aall[j, side].opt()],
                )

            # ---------------- oproj ----------------
            def oth_load(j, side, anchor, engines=None):
                """Gathered per-head tiles for one head-pair: oth[f][:, r, :]
                = global head 4r+off+f of chunk j.  The no-sync dep on
                `anchor` stops the Tile scheduler from hoisting the load to
                right after its AllGather, where its unsatisfied semaphore
                wait would convoy the whole gpsimd queue."""
                aview = aall[j, side][:].rearrange(
                    "(r f p) s -> p r f s", f=2, p=128
                )
                oths = []
                for f in range(2):
                    oth = otp.tile(
                        [128, NCORES, CHUNK], BF16, tag="ot", name=f"oth{f}"
                    )
                    eng = (engines or (nc.gpsimd, nc.gpsimd))[f]
                    for r0 in range(0, NCORES, 2):
                        h = eng.dma_start(
                            out=oth[:, r0 : r0 + 2, :],
                            in_=aview[:, r0 : r0 + 2, f, :],
                        )
                        if anchor is not None:
                            add_dep_helper(
                                h.ins, anchor.ins, sync=False,
                                reason="pin-oth",
                            )
                    oths.append(oth)
                return oths

            def mm8(pso, oth, hglob, qw, first, last):
                for r in range(NCORES):
                    nc.tensor.matmul(
                        pso[:],
                        lhsT=oth[:, r, qw],
                        rhs=wo_bf[:, 4 * r + hglob, :],
                        start=(first and r == 0),
                        stop=(last and r == NCORES - 1),
                        skip_group_check=True,
                    )

            def oproj_store(j, ss, pso):
                osb = ep.tile([128, CHUNK], F32, tag="os", bufs=2)
                if ss >= 2:
                    # late-ss stores land near the attention->projection
                    # boundary where ACT is idle but the DVE still drains
                    # the epilogue; keep them off the DVE queue
                    nc.scalar.copy(out=osb[:], in_=pso[:])
                else:
                    nc.vector.tensor_copy(out=osb[:], in_=pso[:])
                r0 = j * CHUNK + 128 * ss
                nc.sync.dma_start(out=out_d[r0 : r0 + 128, :], in_=osb[:])

            def oproj_quanta(j, oths):
                """16 quanta of 8 matmuls each (one local head x 8 ranks per
                quantum); pso accumulates across the 4 heads of one ss."""
                quanta = []
                for ss in range(CHUNK // 128):
                    qw = bass.ds(128 * ss, 128)
                    box = {}
                    for f in range(4):
                        def qx(ss=ss, qw=qw, box=box, f=f):
                            if f == 0:
                                box["pso"] = ps_prj.tile(
                                    [128, CHUNK], F32, tag="p", name="opso"
                                )
                            mm8(box["pso"], oths[f][:], f, qw,
                                first=(f == 0), last=(f == 3))
                            if f == 3:
                                oproj_store(j, ss, box["pso"])
                        quanta.append(qx)
                return quanta

            def oproj_tail_split(j, oths_a, oths_b):
                """Tail-chunk oproj: phase a (heads 0-1, gated on AG(j,a))
                across all ss with held-open accumulators, then phase b."""
                psos = []
                for ss in range(CHUNK // 128):
                    qw = bass.ds(128 * ss, 128)
                    pso = ps_acc.tile([128, CHUNK], F32, tag="acc")
                    psos.append(pso)
                    for f in range(2):
                        mm8(pso, oths_a[f][:], f, qw,
                            first=(f == 0), last=False)
                for ss in range(CHUNK // 128):
                    qw = bass.ds(128 * ss, 128)
                    pso = psos[ss]
                    for f in range(2):
                        mm8(pso, oths_b[f][:], 2 + f, qw,
                            first=False, last=(f == 1))
                    oproj_store(j, ss, pso)

            # ---------------- attention ----------------
            def attention(j, qT, fillers, skip_pair0_fill=False):
                nk = 4 * (j + 1)  # causal: key tiles 0..nk-1
                pend_dma = None
                fill_it = iter(fillers)

                def fill(n=1):
                    for _ in range(n):
                        f = next(fill_it, None)
                        if f is None:
                            return
                        f()

                def score_block(h, kcs):
                    es = []
                    for kc in kcs:
                        t = kc - 4 * j  # >= 0 on diagonal tiles
                        off = 128 * t if t > 0 else 0
                        pss = ps_sc.tile([128, CHUNK], F32, tag="sc")
                        nc.tensor.matmul(
                            pss[:, off:CHUNK],
                            lhsT=kT_bf[:, 128 * kc : 128 * (kc + 1)],
                            rhs=qT[:, h, off:CHUNK],
                            start=True,
                            stop=True,
                        )
                        e = ep.tile([128, CHUNK], BF16, tag="e", bufs=12)
                        nc.scalar.activation(
                            out=e[:, off:CHUNK], in_=pss[:, off:CHUNK],
                            func=Act.Exp, scale=SCALE,
                        )
                        if t >= 0:
                            # triangular mask on the diagonal 128x128 block
                            nc.gpsimd.affine_select(
                                out=e[:, off : off + 128],
                                in_=e[:, off : off + 128],
                                compare_op=Alu.is_ge,
                                fill=0.0,
                                base=0,
                                channel_multiplier=-1,
                                pattern=[[1, 128]],
                            )
                        es.append((e, off))
                    return es

                def av_block(pso, es, kcs):
                    for (e, off), kc in zip(es, kcs):
                        nc.tensor.matmul(
                            pso[:, off:CHUNK],
                            lhsT=v_bf[:, kc, :],
                            rhs=e[:, off:CHUNK],
                            start=(kc == 0),
                            stop=(kc == nk - 1),
                            skip_group_check=True,
                        )

                def racc_block(racc, es, kcs):
                    for (e, off), kc in zip(es, kcs):
                        if kc == 0:
                            nc.vector.tensor_copy(out=racc[:], in_=e[:])
                        else:
                            nc.vector.tensor_tensor(
                                out=racc[:, off:CHUNK], in0=racc[:, off:CHUNK],
                                in1=e[:, off:CHUNK], op=Alu.add,
                            )

                pending = None
                for hp in range(NH // 2):
                    side = "a" if hp == 0 else "b"
                    h0, h1 = 2 * hp, 2 * hp + 1
                    pso0 = ps_acc.tile([128, CHUNK], F32, tag="acc")
                    pso1 = ps_acc.tile([128, CHUNK], F32, tag="acc")
                    racc0 = ep.tile([128, CHUNK], BF16, tag="racc", bufs=2)
                    racc1 = ep.tile([128, CHUNK], BF16, tag="racc2", bufs=2)
                    # software pipeline: AV(kc-1) is emitted between
                    # score(kc) and score(kc+1) so the PE slots each AV in
                    # right as its exp completes instead of serializing a
                    # whole block of exp-gated scores before any AV
                    prev = None
                    for kc in range(nk):
                        es0 = score_block(h0, [kc])
                        es1 = score_block(h1, [kc])
                        if kc == 3 and pending is not None:
                            # pair0's epilogue lands early in pair1's
                            # stream; its psr/psb matmuls ride the exp
                            # waits -- the AG(a) fires ASAP
                            pend_dma = emit_epilogue(*pending)
                            ag_trigger(pending[0], pending[1])
                            pending = None
                        if prev is not None:
                            p0, p1, pkc = prev
                            av_block(pso0, p0, [pkc])
                            av_block(pso1, p1, [pkc])
                            racc_block(racc0, p0, [pkc])
                            racc_block(racc1, p1, [pkc])
                        if kc % 4 == 2:
                            fill(0 if skip_pair0_fill and hp == 0 else
                                 (2 if skip_pair0_fill else 1))
                        prev = (es0, es1, kc)
                    p0, p1, pkc = prev
                    av_block(pso0, p0, [pkc])
                    av_block(pso1, p1, [pkc])
                    racc_block(racc0, p0, [pkc])
                    racc_block(racc1, p1, [pkc])
                    if pending is not None:  # nk == 4 fallback
                        pend_dma = emit_epilogue(*pending)
                        ag_trigger(pending[0], pending[1])
                        pending = None
                    # free the AV psum banks early: DVE-copy to SBUF so the
                    # epilogue mult reads SBUF and the next chunk's q-accs
                    # (same ring) never wait on the epilogue chain
                    po0 = ep.tile([128, CHUNK], BF16, tag="po", bufs=2)
                    po1 = ep.tile([128, CHUNK], BF16, tag="po2", bufs=2)
                    if hp == NH // 2 - 1:
                        # last pair: ACT is about to go idle (projections
                        # next) and the DVE queue still has the epilogue
                        # chain -- ACT copies free the AV banks faster
                        nc.scalar.copy(out=po0[:], in_=pso0[:])
                        nc.scalar.copy(out=po1[:], in_=pso1[:])
                    else:
                        nc.vector.tensor_copy(out=po0[:], in_=pso0[:])
                        nc.vector.tensor_copy(out=po1[:], in_=pso1[:])
                    pending = (j, side, [(0, po0, racc0), (1, po1, racc1)])
                # pair1 is eager: its AG is the chunk's last collective
                last_dma = emit_epilogue(*pending)
                ag_trigger(pending[0], pending[1])
                fill(len(fillers))  # drain remaining quanta
                return pend_dma, last_dma

            # ---------------- schedule ----------------
            for i0 in range(0, DC, 8):
                if i0 == 0:
                    nc.scalar.dma_start(
                        out=wkv_bf[:, 0:4, :], in_=wkv_v[:, 0:4, :]
                    )
                    nc.scalar.dma_start(
                        out=wkv_bf[:, 4:8, :], in_=wkv_v[:, 4:8, :]
                    )
                else:
                    nc.scalar.dma_start(
                        out=wkv_bf[:, i0 : i0 + 8, :],
                        in_=wkv_v[:, i0 : i0 + 8, :],
                    )
                nc.scalar.dma_start(
                    out=wq_bf[:, i0 : i0 + 4, :], in_=wq_v[:, i0 : i0 + 4, :]
                )
                nc.scalar.dma_start(
                    out=wq_bf[:, i0 + 4 : i0 + 8, :],
                    in_=wq_v[:, i0 + 4 : i0 + 8, :],
                )
            load_weight(wo_bf, wo_v)

            qTs = []
            anchors = []
            oths = {}
            for j in range(NCHUNK):
                qT = qtp.tile([HD, NH, CHUNK], BF16, tag="qT", name=f"qT{j}")
                qTs.append(qT)
                anchors.append(kvq_chunk(j, qT))
                if j >= 2:
                    # oth(j-2) loads, pinned after chunk j's last q-rope so
                    # their AG-done waits are satisfied when the gpsimd
                    # queue reaches them
                    oths[j - 2] = (
                        oth_load(j - 2, "a", anchors[j])
                        + oth_load(j - 2, "b", anchors[j])
                    )
                    a_mid, a_last = attention(
                        j, qT, oproj_quanta(j - 2, oths[j - 2]),
                        skip_pair0_fill=(j == 3),
                    )
                else:
                    a_mid, a_last = attention(j, qT, [])

            # tail: oproj(2) covers AG(3a/3b) wire time, then oproj(3)
            # in two head-pair phases as its AGs land.  The pin anchor is
            # attention(3)'s last ao write, so these loads sit at the very
            # end of the gpsimd stream where their waits block nothing.
            oth2 = oth_load(2, "a", a_mid) + oth_load(2, "b", a_mid)
            oth3a = oth_load(3, "a", a_last)
            oth3b = oth_load(
                3, "b", a_last, engines=(nc.gpsimd, nc.sync)
            )
            for q in oproj_quanta(2, oth2):
                q()
            oproj_tail_split(3, oth3a, oth3b)

    nc.finalize()
    return nc


def _get_graph():
    if "nc" not in _CACHED:
        _CACHED["nc"] = _build_graph()
    return _CACHED["nc"]


def _rope_tables(position_ids):
    pos = np.asarray(position_ids).reshape(-1).astype(np.float64)  # [S]
    inv_freq = 1.0 / (10000.0 ** (np.arange(0, HD, 2, dtype=np.float64) / HD))
    freqs = pos[:, None] * inv_freq[None, :]  # [S, 64]
    emb = np.concatenate([freqs, freqs], axis=-1)  # [S, HD]
    cos_t = np.cos(emb).T.astype(np.float32)  # [HD, S]
    sin_t = np.sin(emb).T.astype(np.float32)
    sin_signed = sin_t.copy()
    sin_signed[: HD // 2] *= -1.0
    bf = ml_dtypes.bfloat16
    return (
        np.ascontiguousarray(cos_t.astype(bf)),
        np.ascontiguousarray(sin_signed.astype(bf)),
    )


def kernel(hidden_states, wq, wk, wv, wo, position_ids, _trace=False):
    bf = ml_dtypes.bfloat16
    hs = np.asarray(hidden_states, np.float32).reshape(S, D)
    hsT = np.ascontiguousarray(hs.T.astype(bf))
    wq = np.asarray(wq, np.float32).astype(bf)
    wk = np.asarray(wk, np.float32).astype(bf)
    wv = np.asarray(wv, np.float32).astype(bf)
    wo = np.asarray(wo, np.float32).astype(bf)
    cos_t, sin_t = _rope_tables(position_ids)

    in_maps = []
    for c in range(NCORES):
        wkv_c = np.concatenate(
            [wk[:, HD * c : HD * (c + 1)], wv[:, HD * c : HD * (c + 1)]],
            axis=1,
        )
        in_maps.append(
            {
                "hsT": hsT,
                "wq": np.ascontiguousarray(wq[:, QCOLS * c : QCOLS * (c + 1)]),
                "wkv": np.ascontiguousarray(wkv_c),
                "wo": np.ascontiguousarray(wo[:, QCOLS * c : QCOLS * (c + 1)]),
                "cos": cos_t,
                "sin": sin_t,
            }
        )

    nc = _get_graph()
    res = run_bass_kernel_spmd(
        nc, in_maps, core_ids=list(range(NCORES)), trace=_trace
    )
    outs = [np.asarray(res.results[c]["out"]) for c in range(NCORES)]
    full = np.concatenate(outs, axis=1).reshape(1, S, D).astype(np.float32)
    if _trace:
        kernel.last_results = res
    return full


# revision 19
# speedup vs baseline: 1.0270x; 1.0270x over previous
"""Distributed Llama-attention Bass kernel for 8 TRN2 NeuronCores.

Sharding: tensor-parallel over heads (core c owns query heads 4c..4c+3 and
KV head c), per-head-pair AllGathers of attention outputs (bf16) pipelined
against later chunks, and a column-shard of wo so each core produces a
disjoint [2048, 512] column slice of the final output (no all-reduce).

v6: single-pass projections + scheduler-pinned gather loads --
  * per chunk, ONE streamed pass over hsT computes k, v and all 4 q heads
    (6 PSUM accumulators): hsT is read once total (16MB instead of 48MB)
    and each 1MB batch carries 12.6us of PE work, so the projection phase
    is compute-dense and the DMA engines never starve the PE;
  * wk/wv arrive host-packed as one [D, 256] tensor (512B DMA lines --
    the split tensors' 256B lines loaded at <80GB/s and stalled startup);
  * AllGathers split per head-pair (8 x 2MB) so the tail oproj can start
    after only half the last chunk's data has arrived;
  * oth gather DMAs are pinned behind a late anchor instruction with a
    no-sync scheduler edge: the Tile scheduler otherwise hoists them to
    right after their AG and their unsatisfied semaphore wait convoys the
    gpsimd queue (v5 lost ~40us per chunk boundary to this);
  * oproj matmuls run as ~2us quanta interleaved into the next-next
    attention's exp-wait bubbles; softmax epilogue uses
    reciprocal_approx_fast; tail: attention(3) -> oproj(2) covers
    AG(3a/3b) wire time -> oproj(3) in two head-pair phases.
"""

import math
import sys

import numpy as np

sys.path.insert(0, "/opt/trn_rl_repo")

import ml_dtypes  # noqa: E402

import concourse.bass as bass  # noqa: E402
import concourse.mybir as mybir  # noqa: E402
import concourse.tile as tile  # noqa: E402
from concourse import bacc  # noqa: E402
from concourse.bass_utils import run_bass_kernel_spmd  # noqa: E402
from concourse.masks import make_identity  # noqa: E402
from concourse.tile import add_dep_helper  # noqa: E402

F32 = mybir.dt.float32
BF16 = mybir.dt.bfloat16
Alu = mybir.AluOpType
Act = mybir.ActivationFunctionType

NCORES = 8
S = 2048
D = 4096
H = 32
HKV = 8
HD = 128
NH = H // NCORES          # 4 local query heads
QCOLS = NH * HD           # 512 local q-proj cols
CHUNK = 512               # s-chunk size
NCHUNK = S // CHUNK       # 4
DC = D // 128             # 32 d-chunks
SCALE = 1.0 / math.sqrt(HD)

_CACHED = {}


def _build_graph():
    nc = bacc.Bacc(
        "TRN2",
        target_bir_lowering=False,
        debug=False,
        num_devices=NCORES,
    )

    hsT_d = nc.dram_tensor("hsT", [D, S], BF16, kind="ExternalInput").ap()
    wq_d = nc.dram_tensor("wq", [D, QCOLS], BF16, kind="ExternalInput").ap()
    wkv_d = nc.dram_tensor("wkv", [D, 2 * HD], BF16, kind="ExternalInput").ap()
    wo_d = nc.dram_tensor("wo", [D, QCOLS], BF16, kind="ExternalInput").ap()
    cos_d = nc.dram_tensor("cos", [HD, S], BF16, kind="ExternalInput").ap()
    sin_d = nc.dram_tensor("sin", [HD, S], BF16, kind="ExternalInput").ap()
    out_d = nc.dram_tensor("out", [S, QCOLS], F32, kind="ExternalOutput").ap()

    hsT_v = hsT_d.rearrange("(i p) s -> p i s", p=128)
    wq_v = wq_d.rearrange("(i p) w -> p i w", p=128)
    wkv_v = wkv_d.rearrange("(i p) w -> p i w", p=128)
    wo_v = wo_d.rearrange("(i p) w -> p i w", p=128)

    with tile.TileContext(nc) as tc:
        with (
            tc.tile_pool(name="persist", bufs=1) as pp,
            tc.tile_pool(name="hsd", bufs=8) as hsdp,
            tc.tile_pool(name="qtp", bufs=2) as qtp,
            tc.tile_pool(name="otp", bufs=6) as otp,
            tc.tile_pool(name="ep", bufs=4) as ep,
            tc.tile_pool(name="rt", bufs=2) as rtp,
            # prj: k/v accumulators + interleaved-oproj psum ring;
            # sc: scores + rowsum/broadcast + v-transpose;
            # acc: q accumulators / AV pairs / tail-oproj accumulators
            tc.tile_pool(name="ps_prj", bufs=2, space="PSUM") as ps_prj,
            tc.tile_pool(name="ps_sc", bufs=2, space="PSUM") as ps_sc,
            tc.tile_pool(name="ps_acc", bufs=4, space="PSUM") as ps_acc,
            tc.tile_pool(name="dram", bufs=1, space="DRAM") as dram,
        ):
            # ---------------- persistent SBUF tensors ----------------
            wq_bf = pp.tile([128, DC, QCOLS], BF16, tag="wq")
            wkv_bf = pp.tile([128, DC, 2 * HD], BF16, tag="wkv")
            wo_bf = pp.tile([128, DC, QCOLS], BF16, tag="wo")
            cos_sb = pp.tile([HD, S], BF16, tag="cos")
            sin_sb = pp.tile([HD, S], BF16, tag="sin")
            kT_bf = pp.tile([HD, S], BF16, tag="kt")
            v_bf = pp.tile([128, S // 128, HD], BF16, tag="v")
            ident = pp.tile([128, 128], BF16, tag="id")
            ones_col = pp.tile([128, 1], BF16, tag="onc")
            ones_row = pp.tile([1, 128], BF16, tag="onr")

            # per-(chunk, head-pair) AllGather buffers: 2 local heads per
            # side -> 256 rows in, 2048 rows gathered
            ain = {}
            aall = {}
            for j in range(NCHUNK):
                for s in ("a", "b"):
                    ain[j, s] = dram.tile(
                        [2 * HD, CHUNK], BF16, tag=f"ain{j}{s}",
                        name=f"ain{j}{s}",
                    )
                    aall[j, s] = dram.tile(
                        [NCORES * 2 * HD, CHUNK], BF16, tag=f"aall{j}{s}",
                        addr_space="Shared", name=f"aall{j}{s}",
                    )

            # ---------------- constants ----------------
            make_identity(nc, ident[:])
            nc.gpsimd.memset(ones_col[:], 1.0)
            nc.gpsimd.memset(ones_row[:], 1.0)
            # cos/sin on the gpsimd queue so they don't delay weight/hs DMA
            nc.gpsimd.dma_start(out=cos_sb[:], in_=cos_d)
            nc.gpsimd.dma_start(out=sin_sb[:], in_=sin_d)

            def load_weight(dst_bf, src_v, n=4):
                for i in range(0, DC, n):
                    nc.scalar.dma_start(
                        out=dst_bf[:, i : i + n, :], in_=src_v[:, i : i + n, :]
                    )

            def rope(psrc, dst_ap, sl):
                t1 = rtp.tile([128, CHUNK], BF16, tag="rt")
                t2 = rtp.tile([128, CHUNK], BF16, tag="rt")
                nc.vector.tensor_tensor(
                    out=t1[0:64, :], in0=psrc[64:128, :],
                    in1=sin_sb[0:64, sl], op=Alu.mult,
                )
                nc.vector.tensor_tensor(
                    out=t1[64:128, :], in0=psrc[0:64, :],
                    in1=sin_sb[64:128, sl], op=Alu.mult,
                )
                nc.vector.tensor_tensor(
                    out=t2[:], in0=psrc[:], in1=cos_sb[:, sl], op=Alu.mult
                )
                return nc.vector.tensor_tensor(
                    out=dst_ap, in0=t1[:], in1=t2[:], op=Alu.add
                )

            # -------- combined K/V/Q projection (one pass per chunk) -----
            def kvq_chunk(j, qT):
                s0 = j * CHUNK
                sl = bass.ds(s0, CHUNK)
                psk = ps_prj.tile([128, CHUNK], F32, tag="p")
                psv = ps_prj.tile([128, CHUNK], F32, tag="p")
                psq = [
                    ps_acc.tile([128, CHUNK], F32, tag="acc", name=f"q{h}")
                    for h in range(NH)
                ]
                NB = 2
                for ib in range(0, DC, NB):
                    hsd = hsdp.tile([128, NB, CHUNK], BF16, tag="hsd")
                    nc.sync.dma_start(
                        out=hsd[:], in_=hsT_v[:, ib : ib + NB, s0 : s0 + CHUNK]
                    )
                    for k in range(NB):
                        i = ib + k
                        st, sp = (i == 0), (i == DC - 1)
                        nc.tensor.matmul(
                            psk[:], lhsT=wkv_bf[:, i, 0:HD], rhs=hsd[:, k, :],
                            start=st, stop=sp,
                        )
                        nc.tensor.matmul(
                            psv[:], lhsT=wkv_bf[:, i, HD : 2 * HD],
                            rhs=hsd[:, k, :], start=st, stop=sp,
                        )
                        for h in range(NH):
                            nc.tensor.matmul(
                                psq[h][:],
                                lhsT=wq_bf[:, i, HD * h : HD * (h + 1)],
                                rhs=hsd[:, k, :], start=st, stop=sp,
                            )
                # drain order matters: the next attention's first score
                # needs qT[h0], then (diagonal tiles, late) kT; v last
                rope(psq[0][:], qT[:, 0, :], sl)
                rope(psq[1][:], qT[:, 1, :], sl)
                rope(psk[:], kT_bf[:, sl], sl)
                vT_sb = ep.tile([128, CHUNK], BF16, tag="vts", bufs=1)
                nc.vector.tensor_copy(out=vT_sb[:], in_=psv[:])
                rope(psq[2][:], qT[:, 2, :], sl)
                anchor = rope(psq[3][:], qT[:, 3, :], sl)
                # v computed as vT [hd, s]; PE-transpose back to [s, hd]
                psv2 = ps_sc.tile([128, 4, 128], BF16, tag="sc")
                for ss in range(CHUNK // 128):
                    nc.tensor.transpose(
                        psv2[:, ss, :],
                        vT_sb[:, 128 * ss : 128 * (ss + 1)],
                        ident[:],
                    )
                nc.vector.tensor_copy(
                    out=v_bf[:, 4 * j : 4 * (j + 1), :], in_=psv2[:]
                )
                return anchor

            # ---------------- softmax epilogue ----------------
            def emit_epilogue(j, side, pairs):
                last_dma = None
                for hloc, pso, racc in pairs:
                    # partition reduce 128 -> 1 with a single ones-matmul
                    psr = ps_sc.tile([1, CHUNK], F32, tag="sc")
                    nc.tensor.matmul(
                        psr[:], lhsT=ones_col[:], rhs=racc[:],
                        start=True, stop=True,
                    )
                    rc32 = ep.tile([1, CHUNK], F32, tag="rc32", bufs=1)
                    nc.vector.reciprocal_approx_fast(out=rc32[:], in_=psr[:])
                    rc = ep.tile([1, CHUNK], BF16, tag="rc", bufs=1)
                    nc.scalar.copy(out=rc[:], in_=rc32[:])
                    psb = ps_sc.tile([128, CHUNK], F32, tag="sc")
                    nc.tensor.matmul(
                        psb[:], lhsT=ones_row[:], rhs=rc[:],
                        start=True, stop=True,
                    )
                    ao = ep.tile([128, CHUNK], BF16, tag="ao", bufs=2)
                    nc.vector.tensor_tensor(
                        out=ao[:], in0=pso[:], in1=psb[:], op=Alu.mult
                    )
                    last_dma = nc.gpsimd.dma_start(
                        out=ain[j, side][HD * hloc : HD * (hloc + 1), :],
                        in_=ao[:],
                    )
                return last_dma

            def ag_trigger(j, side):
                nc.gpsimd.collective_compute(
                    "AllGather",
                    Alu.bypass,
                    replica_groups=[list(range(NCORES))],
                    ins=[ain[j, side].opt()],
                    outs=[aall[j, side].opt()],
                )

            # ---------------- oproj ----------------
            def oth_load(j, side, anchor, engines=None):
                """Gathered per-head tiles for one head-pair: oth[f][:, r, :]
                = global head 4r+off+f of chunk j.  The no-sync dep on
                `anchor` stops the Tile scheduler from hoisting the load to
                right after its AllGather, where its unsatisfied semaphore
                wait would convoy the whole gpsimd queue."""
                aview = aall[j, side][:].rearrange(
                    "(r f p) s -> p r f s", f=2, p=128
                )
                oths = []
                for f in range(2):
                    oth = otp.tile(
                        [128, NCORES, CHUNK], BF16, tag="ot", name=f"oth{f}"
                    )
                    eng = (engines or (nc.gpsimd, nc.gpsimd))[f]
                    for r0 in range(0, NCORES, 2):
                        h = eng.dma_start(
                            out=oth[:, r0 : r0 + 2, :],
                            in_=aview[:, r0 : r0 + 2, f, :],
                        )
                        if anchor is not None:
                            add_dep_helper(
                                h.ins, anchor.ins, sync=False,
                                reason="pin-oth",
                            )
                    oths.append(oth)
                return oths

            def mm8(pso, oth, hglob, qw, first, last):
                for r in range(NCORES):
                    nc.tensor.matmul(
                        pso[:],
                        lhsT=oth[:, r, qw],
                        rhs=wo_bf[:, 4 * r + hglob, :],
                        start=(first and r == 0),
                        stop=(last and r == NCORES - 1),
                        skip_group_check=True,
                    )

            def oproj_store(j, ss, pso):
                osb = ep.tile([128, CHUNK], F32, tag="os", bufs=2)
                nc.vector.tensor_copy(out=osb[:], in_=pso[:])
                r0 = j * CHUNK + 128 * ss
                nc.sync.dma_start(out=out_d[r0 : r0 + 128, :], in_=osb[:])

            def oproj_quanta(j, oths):
                """16 quanta of 8 matmuls each (one local head x 8 ranks per
                quantum); pso accumulates across the 4 heads of one ss."""
                quanta = []
                for ss in range(CHUNK // 128):
                    qw = bass.ds(128 * ss, 128)
                    box = {}
                    for f in range(4):
                        def qx(ss=ss, qw=qw, box=box, f=f):
                            if f == 0:
                                box["pso"] = ps_prj.tile(
                                    [128, CHUNK], F32, tag="p", name="opso"
                                )
                            mm8(box["pso"], oths[f][:], f, qw,
                                first=(f == 0), last=(f == 3))
                            if f == 3:
                                oproj_store(j, ss, box["pso"])
                        quanta.append(qx)
                return quanta

            def oproj_tail_split(j, oths_a, oths_b):
                """Tail-chunk oproj: phase a (heads 0-1, gated on AG(j,a))
                across all ss with held-open accumulators, then phase b."""
                psos = []
                for ss in range(CHUNK // 128):
                    qw = bass.ds(128 * ss, 128)
                    pso = ps_acc.tile([128, CHUNK], F32, tag="acc")
                    psos.append(pso)
                    for f in range(2):
                        mm8(pso, oths_a[f][:], f, qw,
                            first=(f == 0), last=False)
                for ss in range(CHUNK // 128):
                    qw = bass.ds(128 * ss, 128)
                    pso = psos[ss]
                    for f in range(2):
                        mm8(pso, oths_b[f][:], 2 + f, qw,
                            first=False, last=(f == 1))
                    oproj_store(j, ss, pso)

            # ---------------- attention ----------------
            def attention(j, qT, fillers, skip_pair0_fill=False):
                nk = 4 * (j + 1)  # causal: key tiles 0..nk-1
                pend_dma = None
                fill_it = iter(fillers)

                def fill(n=1):
                    for _ in range(n):
                        f = next(fill_it, None)
                        if f is None:
                            return
                        f()

                def score_block(h, kcs):
                    es = []
                    for kc in kcs:
                        t = kc - 4 * j  # >= 0 on diagonal tiles
                        off = 128 * t if t > 0 else 0
                        pss = ps_sc.tile([128, CHUNK], F32, tag="sc")
                        nc.tensor.matmul(
                            pss[:, off:CHUNK],
                            lhsT=kT_bf[:, 128 * kc : 128 * (kc + 1)],
                            rhs=qT[:, h, off:CHUNK],
                            start=True,
                            stop=True,
                        )
                        e = ep.tile([128, CHUNK], BF16, tag="e", bufs=12)
                        nc.scalar.activation(
                            out=e[:, off:CHUNK], in_=pss[:, off:CHUNK],
                            func=Act.Exp, scale=SCALE,
                        )
                        if t >= 0:
                            # triangular mask on the diagonal 128x128 block
                            nc.gpsimd.affine_select(
                                out=e[:, off : off + 128],
                                in_=e[:, off : off + 128],
                                compare_op=Alu.is_ge,
                                fill=0.0,
                                base=0,
                                channel_multiplier=-1,
                                pattern=[[1, 128]],
                            )
                        es.append((e, off))
                    return es

                def av_block(pso, es, kcs):
                    for (e, off), kc in zip(es, kcs):
                        nc.tensor.matmul(
                            pso[:, off:CHUNK],
                            lhsT=v_bf[:, kc, :],
                            rhs=e[:, off:CHUNK],
                            start=(kc == 0),
                            stop=(kc == nk - 1),
                            skip_group_check=True,
                        )

                def racc_block(racc, es, kcs):
                    for (e, off), kc in zip(es, kcs):
                        if kc == 0:
                            nc.vector.tensor_copy(out=racc[:], in_=e[:])
                        else:
                            nc.vector.tensor_tensor(
                                out=racc[:, off:CHUNK], in0=racc[:, off:CHUNK],
                                in1=e[:, off:CHUNK], op=Alu.add,
                            )

                pending = None
                for hp in range(NH // 2):
                    side = "a" if hp == 0 else "b"
                    h0, h1 = 2 * hp, 2 * hp + 1
                    pso0 = ps_acc.tile([128, CHUNK], F32, tag="acc")
                    pso1 = ps_acc.tile([128, CHUNK], F32, tag="acc")
                    racc0 = ep.tile([128, CHUNK], BF16, tag="racc", bufs=2)
                    racc1 = ep.tile([128, CHUNK], BF16, tag="racc2", bufs=2)
                    # software pipeline: AV(kc-1) is emitted between
                    # score(kc) and score(kc+1) so the PE slots each AV in
                    # right as its exp completes instead of serializing a
                    # whole block of exp-gated scores before any AV
                    prev = None
                    for kc in range(nk):
                        es0 = score_block(h0, [kc])
                        es1 = score_block(h1, [kc])
                        if kc == 3 and pending is not None:
                            # pair0's epilogue lands early in pair1's
                            # stream; its psr/psb matmuls ride the exp
                            # waits -- the AG(a) fires ASAP
                            pend_dma = emit_epilogue(*pending)
                            ag_trigger(pending[0], pending[1])
                            pending = None
                        if prev is not None:
                            p0, p1, pkc = prev
                            av_block(pso0, p0, [pkc])
                            av_block(pso1, p1, [pkc])
                            racc_block(racc0, p0, [pkc])
                            racc_block(racc1, p1, [pkc])
                        if kc % 4 == 2:
                            fill(0 if skip_pair0_fill and hp == 0 else
                                 (2 if skip_pair0_fill else 1))
                        prev = (es0, es1, kc)
                    p0, p1, pkc = prev
                    av_block(pso0, p0, [pkc])
                    av_block(pso1, p1, [pkc])
                    racc_block(racc0, p0, [pkc])
                    racc_block(racc1, p1, [pkc])
                    if pending is not None:  # nk == 4 fallback
                        pend_dma = emit_epilogue(*pending)
                        ag_trigger(pending[0], pending[1])
                        pending = None
                    # free the AV psum banks early: DVE-copy to SBUF so the
                    # epilogue mult reads SBUF and the next chunk's q-accs
                    # (same ring) never wait on the epilogue chain
                    po0 = ep.tile([128, CHUNK], BF16, tag="po", bufs=2)
                    po1 = ep.tile([128, CHUNK], BF16, tag="po2", bufs=2)
                    if hp == NH // 2 - 1:
                        # last pair: ACT is about to go idle (projections
                        # next) and the DVE queue still has the epilogue
                        # chain -- ACT copies free the AV banks faster
                        nc.scalar.copy(out=po0[:], in_=pso0[:])
                        nc.scalar.copy(out=po1[:], in_=pso1[:])
                    else:
                        nc.vector.tensor_copy(out=po0[:], in_=pso0[:])
                        nc.vector.tensor_copy(out=po1[:], in_=pso1[:])
                    pending = (j, side, [(0, po0, racc0), (1, po1, racc1)])
                # pair1 is eager: its AG is the chunk's last collective
                last_dma = emit_epilogue(*pending)
                ag_trigger(pending[0], pending[1])
                fill(len(fillers))  # drain remaining quanta
                return pend_dma, last_dma

            # ---------------- schedule ----------------
            for i0 in range(0, DC, 8):
                if i0 == 0:
                    nc.scalar.dma_start(
                        out=wkv_bf[:, 0:4, :], in_=wkv_v[:, 0:4, :]
                    )
                    nc.scalar.dma_start(
                        out=wkv_bf[:, 4:8, :], in_=wkv_v[:, 4:8, :]
                    )
                else:
                    nc.scalar.dma_start(
                        out=wkv_bf[:, i0 : i0 + 8, :],
                        in_=wkv_v[:, i0 : i0 + 8, :],
                    )
                nc.scalar.dma_start(
                    out=wq_bf[:, i0 : i0 + 4, :], in_=wq_v[:, i0 : i0 + 4, :]
                )
                nc.scalar.dma_start(
                    out=wq_bf[:, i0 + 4 : i0 + 8, :],
                    in_=wq_v[:, i0 + 4 : i0 + 8, :],
                )
            load_weight(wo_bf, wo_v)

            qTs = []
            anchors = []
            oths = {}
            for j in range(NCHUNK):
                qT = qtp.tile([HD, NH, CHUNK], BF16, tag="qT", name=f"qT{j}")
                qTs.append(qT)
                anchors.append(kvq_chunk(j, qT))
                if j >= 2:
                    # oth(j-2) loads, pinned after chunk j's last q-rope so
                    # their AG-done waits are satisfied when the gpsimd
                    # queue reaches them
                    oths[j - 2] = (
                        oth_load(j - 2, "a", anchors[j])
                        + oth_load(j - 2, "b", anchors[j])
                    )
                    a_mid, a_last = attention(
                        j, qT, oproj_quanta(j - 2, oths[j - 2]),
                        skip_pair0_fill=(j == 3),
                    )
                else:
                    a_mid, a_last = attention(j, qT, [])

            # tail: oproj(2) covers AG(3a/3b) wire time, then oproj(3)
            # in two head-pair phases as its AGs land.  The pin anchor is
            # attention(3)'s last ao write, so these loads sit at the very
            # end of the gpsimd stream where their waits block nothing.
            oth2 = oth_load(2, "a", a_mid) + oth_load(2, "b", a_mid)
            oth3a = oth_load(3, "a", a_last)
            oth3b = oth_load(
                3, "b", a_last, engines=(nc.gpsimd, nc.sync)
            )
            for q in oproj_quanta(2, oth2):
                q()
            oproj_tail_split(3, oth3a, oth3b)

    nc.finalize()
    return nc


def _get_graph():
    if "nc" not in _CACHED:
        _CACHED["nc"] = _build_graph()
    return _CACHED["nc"]


def _rope_tables(position_ids):
    pos = np.asarray(position_ids).reshape(-1).astype(np.float64)  # [S]
    inv_freq = 1.0 / (10000.0 ** (np.arange(0, HD, 2, dtype=np.float64) / HD))
    freqs = pos[:, None] * inv_freq[None, :]  # [S, 64]
    emb = np.concatenate([freqs, freqs], axis=-1)  # [S, HD]
    cos_t = np.cos(emb).T.astype(np.float32)  # [HD, S]
    sin_t = np.sin(emb).T.astype(np.float32)
    sin_signed = sin_t.copy()
    sin_signed[: HD // 2] *= -1.0
    bf = ml_dtypes.bfloat16
    return (
        np.ascontiguousarray(cos_t.astype(bf)),
        np.ascontiguousarray(sin_signed.astype(bf)),
    )


def kernel(hidden_states, wq, wk, wv, wo, position_ids, _trace=False):
    bf = ml_dtypes.bfloat16
    hs = np.asarray(hidden_states, np.float32).reshape(S, D)
    hsT = np.ascontiguousarray(hs.T.astype(bf))
    wq = np.asarray(wq, np.float32).astype(bf)
    wk = np.asarray(wk, np.float32).astype(bf)
    wv = np.asarray(wv, np.float32).astype(bf)
    wo = np.asarray(wo, np.float32).astype(bf)
    cos_t, sin_t = _rope_tables(position_ids)

    in_maps = []
    for c in range(NCORES):
        wkv_c = np.concatenate(
            [wk[:, HD * c : HD * (c + 1)], wv[:, HD * c : HD * (c + 1)]],
            axis=1,
        )
        in_maps.append(
            {
                "hsT": hsT,
                "wq": np.ascontiguousarray(wq[:, QCOLS * c : QCOLS * (c + 1)]),
                "wkv": np.ascontiguousarray(wkv_c),
                "wo": np.ascontiguousarray(wo[:, QCOLS * c : QCOLS * (c + 1)]),
                "cos": cos_t,
                "sin": sin_t,
            }
        )

    nc = _get_graph()
    res = run_bass_kernel_spmd(
        nc, in_maps, core_ids=list(range(NCORES)), trace=_trace
    )
    outs = [np.asarray(res.results[c]["out"]) for c in range(NCORES)]
    full = np.concatenate(outs, axis=1).reshape(1, S, D).astype(np.float32)
    if _trace:
        kernel.last_results = res
    return full


# revision 20
# speedup vs baseline: 1.0556x; 1.0279x over previous
"""Distributed Llama-attention Bass kernel for 8 TRN2 NeuronCores.

Sharding: tensor-parallel over heads (core c owns query heads 4c..4c+3 and
KV head c), per-head-pair AllGathers of attention outputs (bf16) pipelined
against later chunks, and a column-shard of wo so each core produces a
disjoint [2048, 512] column slice of the final output (no all-reduce).

v6: single-pass projections + scheduler-pinned gather loads --
  * per chunk, ONE streamed pass over hsT computes k, v and all 4 q heads
    (6 PSUM accumulators): hsT is read once total (16MB instead of 48MB)
    and each 1MB batch carries 12.6us of PE work, so the projection phase
    is compute-dense and the DMA engines never starve the PE;
  * wk/wv arrive host-packed as one [D, 256] tensor (512B DMA lines --
    the split tensors' 256B lines loaded at <80GB/s and stalled startup);
  * AllGathers split per head-pair (8 x 2MB) so the tail oproj can start
    after only half the last chunk's data has arrived;
  * oth gather DMAs are pinned behind a late anchor instruction with a
    no-sync scheduler edge: the Tile scheduler otherwise hoists them to
    right after their AG and their unsatisfied semaphore wait convoys the
    gpsimd queue (v5 lost ~40us per chunk boundary to this);
  * oproj matmuls run as ~2us quanta interleaved into the next-next
    attention's exp-wait bubbles; softmax epilogue uses
    reciprocal_approx_fast; tail: attention(3) -> oproj(2) covers
    AG(3a/3b) wire time -> oproj(3) in two head-pair phases.
"""

import math
import sys

import numpy as np

sys.path.insert(0, "/opt/trn_rl_repo")

import ml_dtypes  # noqa: E402

import concourse.bass as bass  # noqa: E402
import concourse.mybir as mybir  # noqa: E402
import concourse.tile as tile  # noqa: E402
from concourse import bacc  # noqa: E402
from concourse.bass_utils import run_bass_kernel_spmd  # noqa: E402
from concourse.masks import make_identity  # noqa: E402
from concourse.tile import add_dep_helper  # noqa: E402

F32 = mybir.dt.float32
BF16 = mybir.dt.bfloat16
Alu = mybir.AluOpType
Act = mybir.ActivationFunctionType

NCORES = 8
S = 2048
D = 4096
H = 32
HKV = 8
HD = 128
NH = H // NCORES          # 4 local query heads
QCOLS = NH * HD           # 512 local q-proj cols
CHUNK = 512               # s-chunk size
NCHUNK = S // CHUNK       # 4
DC = D // 128             # 32 d-chunks
SCALE = 1.0 / math.sqrt(HD)

_CACHED = {}


def _build_graph():
    nc = bacc.Bacc(
        "TRN2",
        target_bir_lowering=False,
        debug=False,
        num_devices=NCORES,
    )

    hsT_d = nc.dram_tensor("hsT", [D, S], BF16, kind="ExternalInput").ap()
    wq_d = nc.dram_tensor("wq", [D, QCOLS], BF16, kind="ExternalInput").ap()
    wkv_d = nc.dram_tensor("wkv", [D, 2 * HD], BF16, kind="ExternalInput").ap()
    wo_d = nc.dram_tensor("wo", [D, QCOLS], BF16, kind="ExternalInput").ap()
    cos_d = nc.dram_tensor("cos", [HD, S], BF16, kind="ExternalInput").ap()
    sin_d = nc.dram_tensor("sin", [HD, S], BF16, kind="ExternalInput").ap()
    out_d = nc.dram_tensor("out", [S, QCOLS], F32, kind="ExternalOutput").ap()

    hsT_v = hsT_d.rearrange("(i p) s -> p i s", p=128)
    wq_v = wq_d.rearrange("(i p) w -> p i w", p=128)
    wkv_v = wkv_d.rearrange("(i p) w -> p i w", p=128)
    wo_v = wo_d.rearrange("(i p) w -> p i w", p=128)

    with tile.TileContext(nc) as tc:
        with (
            tc.tile_pool(name="persist", bufs=1) as pp,
            tc.tile_pool(name="hsd", bufs=8) as hsdp,
            tc.tile_pool(name="qtp", bufs=2) as qtp,
            tc.tile_pool(name="otp", bufs=6) as otp,
            tc.tile_pool(name="ep", bufs=4) as ep,
            tc.tile_pool(name="rt", bufs=2) as rtp,
            # prj: k/v accumulators + interleaved-oproj psum ring;
            # sc: scores + rowsum/broadcast + v-transpose;
            # acc: q accumulators / AV pairs / tail-oproj accumulators
            tc.tile_pool(name="ps_prj", bufs=2, space="PSUM") as ps_prj,
            tc.tile_pool(name="ps_sc", bufs=2, space="PSUM") as ps_sc,
            tc.tile_pool(name="ps_acc", bufs=4, space="PSUM") as ps_acc,
            tc.tile_pool(name="dram", bufs=1, space="DRAM") as dram,
        ):
            # ---------------- persistent SBUF tensors ----------------
            wq_bf = pp.tile([128, DC, QCOLS], BF16, tag="wq")
            wkv_bf = pp.tile([128, DC, 2 * HD], BF16, tag="wkv")
            wo_bf = pp.tile([128, DC, QCOLS], BF16, tag="wo")
            cos_sb = pp.tile([HD, S], BF16, tag="cos")
            sin_sb = pp.tile([HD, S], BF16, tag="sin")
            kT_bf = pp.tile([HD, S], BF16, tag="kt")
            v_bf = pp.tile([128, S // 128, HD], BF16, tag="v")
            ident = pp.tile([128, 128], BF16, tag="id")
            ones_col = pp.tile([128, 1], BF16, tag="onc")
            ones_row = pp.tile([1, 128], BF16, tag="onr")

            # per-(chunk, head-pair) AllGather buffers: 2 local heads per
            # side -> 256 rows in, 2048 rows gathered
            ain = {}
            aall = {}
            for j in range(NCHUNK):
                for s in ("a", "b"):
                    ain[j, s] = dram.tile(
                        [2 * HD, CHUNK], BF16, tag=f"ain{j}{s}",
                        name=f"ain{j}{s}",
                    )
                    aall[j, s] = dram.tile(
                        [NCORES * 2 * HD, CHUNK], BF16, tag=f"aall{j}{s}",
                        addr_space="Shared", name=f"aall{j}{s}",
                    )

            # ---------------- constants ----------------
            make_identity(nc, ident[:])
            nc.gpsimd.memset(ones_col[:], 1.0)
            nc.gpsimd.memset(ones_row[:], 1.0)
            # cos/sin on the gpsimd queue so they don't delay weight/hs DMA
            nc.gpsimd.dma_start(out=cos_sb[:], in_=cos_d)
            nc.gpsimd.dma_start(out=sin_sb[:], in_=sin_d)

            # warm-up: the first weight DMA takes 10-19us to land (cold
            # device) while the PE idles, which also leaves the HAM clock
            # gate at 1.2GHz into the first projection chunk.  Dummy
            # identity matmuls (never read) keep the array busy and warm;
            # 90 finish before the earliest observed weight arrival.
            warm = ps_sc.tile([128, CHUNK], F32, tag="sc", name="warm")
            for _ in range(90):
                nc.tensor.matmul(
                    warm[:, 0:128], lhsT=ident[:], rhs=ident[:],
                    start=True, stop=True,
                )

            def load_weight(dst_bf, src_v, n=4):
                for i in range(0, DC, n):
                    nc.scalar.dma_start(
                        out=dst_bf[:, i : i + n, :], in_=src_v[:, i : i + n, :]
                    )

            def rope(psrc, dst_ap, sl):
                t1 = rtp.tile([128, CHUNK], BF16, tag="rt")
                t2 = rtp.tile([128, CHUNK], BF16, tag="rt")
                nc.vector.tensor_tensor(
                    out=t1[0:64, :], in0=psrc[64:128, :],
                    in1=sin_sb[0:64, sl], op=Alu.mult,
                )
                nc.vector.tensor_tensor(
                    out=t1[64:128, :], in0=psrc[0:64, :],
                    in1=sin_sb[64:128, sl], op=Alu.mult,
                )
                nc.vector.tensor_tensor(
                    out=t2[:], in0=psrc[:], in1=cos_sb[:, sl], op=Alu.mult
                )
                return nc.vector.tensor_tensor(
                    out=dst_ap, in0=t1[:], in1=t2[:], op=Alu.add
                )

            # -------- combined K/V/Q projection (one pass per chunk) -----
            def kvq_chunk(j, qT):
                s0 = j * CHUNK
                sl = bass.ds(s0, CHUNK)
                psk = ps_prj.tile([128, CHUNK], F32, tag="p")
                psv = ps_prj.tile([128, CHUNK], F32, tag="p")
                psq = [
                    ps_acc.tile([128, CHUNK], F32, tag="acc", name=f"q{h}")
                    for h in range(NH)
                ]
                NB = 2
                hsds = {}
                for ib in range(0, DC, NB):
                    hsd = hsdp.tile([128, NB, CHUNK], BF16, tag="hsd")
                    nc.sync.dma_start(
                        out=hsd[:], in_=hsT_v[:, ib : ib + NB, s0 : s0 + CHUNK]
                    )
                    hsds[ib] = hsd
                    if ib < DC - 4:
                        for k in range(NB):
                            i = ib + k
                            st = (i == 0)
                            nc.tensor.matmul(
                                psk[:], lhsT=wkv_bf[:, i, 0:HD],
                                rhs=hsd[:, k, :], start=st, stop=False,
                            )
                            nc.tensor.matmul(
                                psv[:], lhsT=wkv_bf[:, i, HD : 2 * HD],
                                rhs=hsd[:, k, :], start=st, stop=False,
                            )
                            for h in range(NH):
                                nc.tensor.matmul(
                                    psq[h][:],
                                    lhsT=wq_bf[:, i, HD * h : HD * (h + 1)],
                                    rhs=hsd[:, k, :], start=st, stop=False,
                                )
                    elif ib == DC - 2:
                        # last 4 d-chunks accumulator-major with q0/q1
                        # first: their stop fires ~5us before the stream
                        # ends, so the q-ropes (4 serial DVE ops each)
                        # overlap the remaining matmuls and the next
                        # attention's first scores start without waiting
                        order = (
                            [(wq_bf, HD * h, HD * (h + 1), psq[h])
                             for h in (0, 1)]
                            + [(wkv_bf, 0, HD, psk),
                               (wkv_bf, HD, 2 * HD, psv)]
                            + [(wq_bf, HD * h, HD * (h + 1), psq[h])
                               for h in (2, 3)]
                        )
                        for w, c0, c1, acc in order:
                            for i in range(DC - 4, DC):
                                b0 = DC - 4 if i < DC - 2 else DC - 2
                                nc.tensor.matmul(
                                    acc[:], lhsT=w[:, i, c0:c1],
                                    rhs=hsds[b0][:, i - b0, :],
                                    start=False, stop=(i == DC - 1),
                                )
                # drain order matters: the next attention's first score
                # needs qT[h0], then (diagonal tiles, late) kT; v last
                rope(psq[0][:], qT[:, 0, :], sl)
                rope(psq[1][:], qT[:, 1, :], sl)
                rope(psk[:], kT_bf[:, sl], sl)
                vT_sb = ep.tile([128, CHUNK], BF16, tag="vts", bufs=1)
                nc.vector.tensor_copy(out=vT_sb[:], in_=psv[:])
                rope(psq[2][:], qT[:, 2, :], sl)
                anchor = rope(psq[3][:], qT[:, 3, :], sl)
                # v computed as vT [hd, s]; PE-transpose back to [s, hd]
                psv2 = ps_sc.tile([128, 4, 128], BF16, tag="sc")
                for ss in range(CHUNK // 128):
                    nc.tensor.transpose(
                        psv2[:, ss, :],
                        vT_sb[:, 128 * ss : 128 * (ss + 1)],
                        ident[:],
                    )
                nc.vector.tensor_copy(
                    out=v_bf[:, 4 * j : 4 * (j + 1), :], in_=psv2[:]
                )
                return anchor

            # ---------------- softmax epilogue ----------------
            def emit_epilogue(j, side, pairs):
                last_dma = None
                for hloc, pso, racc in pairs:
                    # partition reduce 128 -> 1 with a single ones-matmul
                    psr = ps_sc.tile([1, CHUNK], F32, tag="sc")
                    nc.tensor.matmul(
                        psr[:], lhsT=ones_col[:], rhs=racc[:],
                        start=True, stop=True,
                    )
                    rc32 = ep.tile([1, CHUNK], F32, tag="rc32", bufs=1)
                    nc.vector.reciprocal_approx_fast(out=rc32[:], in_=psr[:])
                    rc = ep.tile([1, CHUNK], BF16, tag="rc", bufs=1)
                    nc.scalar.copy(out=rc[:], in_=rc32[:])
                    psb = ps_sc.tile([128, CHUNK], F32, tag="sc")
                    nc.tensor.matmul(
                        psb[:], lhsT=ones_row[:], rhs=rc[:],
                        start=True, stop=True,
                    )
                    ao = ep.tile([128, CHUNK], BF16, tag="ao", bufs=2)
                    nc.vector.tensor_tensor(
                        out=ao[:], in0=pso[:], in1=psb[:], op=Alu.mult
                    )
                    last_dma = nc.gpsimd.dma_start(
                        out=ain[j, side][HD * hloc : HD * (hloc + 1), :],
                        in_=ao[:],
                    )
                return last_dma

            def ag_trigger(j, side):
                nc.gpsimd.collective_compute(
                    "AllGather",
                    Alu.bypass,
                    replica_groups=[list(range(NCORES))],
                    ins=[ain[j, side].opt()],
                    outs=[aall[j, side].opt()],
                )

            # ---------------- oproj ----------------
            def oth_load(j, side, anchor, engines=None):
                """Gathered per-head tiles for one head-pair: oth[f][:, r, :]
                = global head 4r+off+f of chunk j.  The no-sync dep on
                `anchor` stops the Tile scheduler from hoisting the load to
                right after its AllGather, where its unsatisfied semaphore
                wait would convoy the whole gpsimd queue."""
                aview = aall[j, side][:].rearrange(
                    "(r f p) s -> p r f s", f=2, p=128
                )
                oths = []
                for f in range(2):
                    oth = otp.tile(
                        [128, NCORES, CHUNK], BF16, tag="ot", name=f"oth{f}"
                    )
                    eng = (engines or (nc.gpsimd, nc.gpsimd))[f]
                    for r0 in range(0, NCORES, 2):
                        h = eng.dma_start(
                            out=oth[:, r0 : r0 + 2, :],
                            in_=aview[:, r0 : r0 + 2, f, :],
                        )
                        if anchor is not None:
                            add_dep_helper(
                                h.ins, anchor.ins, sync=False,
                                reason="pin-oth",
                            )
                    oths.append(oth)
                return oths

            def mm8(pso, oth, hglob, qw, first, last):
                for r in range(NCORES):
                    nc.tensor.matmul(
                        pso[:],
                        lhsT=oth[:, r, qw],
                        rhs=wo_bf[:, 4 * r + hglob, :],
                        start=(first and r == 0),
                        stop=(last and r == NCORES - 1),
                        skip_group_check=True,
                    )

            def oproj_store(j, ss, pso):
                osb = ep.tile([128, CHUNK], F32, tag="os", bufs=2)
                nc.vector.tensor_copy(out=osb[:], in_=pso[:])
                r0 = j * CHUNK + 128 * ss
                nc.sync.dma_start(out=out_d[r0 : r0 + 128, :], in_=osb[:])

            def oproj_quanta(j, oths):
                """16 quanta of 8 matmuls each (one local head x 8 ranks per
                quantum); pso accumulates across the 4 heads of one ss."""
                quanta = []
                for ss in range(CHUNK // 128):
                    qw = bass.ds(128 * ss, 128)
                    box = {}
                    for f in range(4):
                        def qx(ss=ss, qw=qw, box=box, f=f):
                            if f == 0:
                                box["pso"] = ps_prj.tile(
                                    [128, CHUNK], F32, tag="p", name="opso"
                                )
                            mm8(box["pso"], oths[f][:], f, qw,
                                first=(f == 0), last=(f == 3))
                            if f == 3:
                                oproj_store(j, ss, box["pso"])
                        quanta.append(qx)
                return quanta

            def oproj_tail_split(j, oths_a, oths_b):
                """Tail-chunk oproj: phase a (heads 0-1, gated on AG(j,a))
                across all ss with held-open accumulators, then phase b."""
                psos = []
                for ss in range(CHUNK // 128):
                    qw = bass.ds(128 * ss, 128)
                    pso = ps_acc.tile([128, CHUNK], F32, tag="acc")
                    psos.append(pso)
                    for f in range(2):
                        mm8(pso, oths_a[f][:], f, qw,
                            first=(f == 0), last=False)
                for ss in range(CHUNK // 128):
                    qw = bass.ds(128 * ss, 128)
                    pso = psos[ss]
                    for f in range(2):
                        mm8(pso, oths_b[f][:], 2 + f, qw,
                            first=False, last=(f == 1))
                    oproj_store(j, ss, pso)

            # ---------------- attention ----------------
            def attention(j, qT, fillers, skip_pair0_fill=False):
                nk = 4 * (j + 1)  # causal: key tiles 0..nk-1
                pend_dma = None
                fill_it = iter(fillers)

                def fill(n=1):
                    for _ in range(n):
                        f = next(fill_it, None)
                        if f is None:
                            return
                        f()

                def score_block(h, kcs):
                    es = []
                    for kc in kcs:
                        t = kc - 4 * j  # >= 0 on diagonal tiles
                        off = 128 * t if t > 0 else 0
                        pss = ps_sc.tile([128, CHUNK], F32, tag="sc")
                        nc.tensor.matmul(
                            pss[:, off:CHUNK],
                            lhsT=kT_bf[:, 128 * kc : 128 * (kc + 1)],
                            rhs=qT[:, h, off:CHUNK],
                            start=True,
                            stop=True,
                        )
                        e = ep.tile([128, CHUNK], BF16, tag="e", bufs=12)
                        nc.scalar.activation(
                            out=e[:, off:CHUNK], in_=pss[:, off:CHUNK],
                            func=Act.Exp, scale=SCALE,
                        )
                        if t >= 0:
                            # triangular mask on the diagonal 128x128 block
                            nc.gpsimd.affine_select(
                                out=e[:, off : off + 128],
                                in_=e[:, off : off + 128],
                                compare_op=Alu.is_ge,
                                fill=0.0,
                                base=0,
                                channel_multiplier=-1,
                                pattern=[[1, 128]],
                            )
                        es.append((e, off))
                    return es

                def av_block(pso, es, kcs):
                    for (e, off), kc in zip(es, kcs):
                        nc.tensor.matmul(
                            pso[:, off:CHUNK],
                            lhsT=v_bf[:, kc, :],
                            rhs=e[:, off:CHUNK],
                            start=(kc == 0),
                            stop=(kc == nk - 1),
                            skip_group_check=True,
                        )

                def racc_block(racc, es, kcs):
                    for (e, off), kc in zip(es, kcs):
                        if kc == 0:
                            nc.vector.tensor_copy(out=racc[:], in_=e[:])
                        else:
                            nc.vector.tensor_tensor(
                                out=racc[:, off:CHUNK], in0=racc[:, off:CHUNK],
                                in1=e[:, off:CHUNK], op=Alu.add,
                            )

                pending = None
                for hp in range(NH // 2):
                    side = "a" if hp == 0 else "b"
                    h0, h1 = 2 * hp, 2 * hp + 1
                    pso0 = ps_acc.tile([128, CHUNK], F32, tag="acc")
                    pso1 = ps_acc.tile([128, CHUNK], F32, tag="acc")
                    racc0 = ep.tile([128, CHUNK], BF16, tag="racc", bufs=2)
                    racc1 = ep.tile([128, CHUNK], BF16, tag="racc2", bufs=2)
                    # software pipeline: AV(kc-1) is emitted between
                    # score(kc) and score(kc+1) so the PE slots each AV in
                    # right as its exp completes instead of serializing a
                    # whole block of exp-gated scores before any AV
                    prev = None
                    for kc in range(nk):
                        es0 = score_block(h0, [kc])
                        es1 = score_block(h1, [kc])
                        if kc == 3 and pending is not None:
                            # pair0's epilogue lands early in pair1's
                            # stream; its psr/psb matmuls ride the exp
                            # waits -- the AG(a) fires ASAP
                            pend_dma = emit_epilogue(*pending)
                            ag_trigger(pending[0], pending[1])
                            pending = None
                        if prev is not None:
                            p0, p1, pkc = prev
                            av_block(pso0, p0, [pkc])
                            av_block(pso1, p1, [pkc])
                            racc_block(racc0, p0, [pkc])
                            racc_block(racc1, p1, [pkc])
                        if kc % 4 == 2:
                            fill(0 if skip_pair0_fill and hp == 0 else
                                 (2 if skip_pair0_fill else 1))
                        prev = (es0, es1, kc)
                    p0, p1, pkc = prev
                    av_block(pso0, p0, [pkc])
                    av_block(pso1, p1, [pkc])
                    racc_block(racc0, p0, [pkc])
                    racc_block(racc1, p1, [pkc])
                    if pending is not None:  # nk == 4 fallback
                        pend_dma = emit_epilogue(*pending)
                        ag_trigger(pending[0], pending[1])
                        pending = None
                    # free the AV psum banks early: DVE-copy to SBUF so the
                    # epilogue mult reads SBUF and the next chunk's q-accs
                    # (same ring) never wait on the epilogue chain
                    po0 = ep.tile([128, CHUNK], BF16, tag="po", bufs=2)
                    po1 = ep.tile([128, CHUNK], BF16, tag="po2", bufs=2)
                    if hp == NH // 2 - 1:
                        # last pair: ACT is about to go idle (projections
                        # next) and the DVE queue still has the epilogue
                        # chain -- ACT copies free the AV banks faster
                        nc.scalar.copy(out=po0[:], in_=pso0[:])
                        nc.scalar.copy(out=po1[:], in_=pso1[:])
                    else:
                        nc.vector.tensor_copy(out=po0[:], in_=pso0[:])
                        nc.vector.tensor_copy(out=po1[:], in_=pso1[:])
                    pending = (j, side, [(0, po0, racc0), (1, po1, racc1)])
                # pair1 is eager: its AG is the chunk's last collective
                last_dma = emit_epilogue(*pending)
                ag_trigger(pending[0], pending[1])
                fill(len(fillers))  # drain remaining quanta
                return pend_dma, last_dma

            # ---------------- schedule ----------------
            for i0 in range(0, DC, 8):
                if i0 == 0:
                    nc.scalar.dma_start(
                        out=wkv_bf[:, 0:4, :], in_=wkv_v[:, 0:4, :]
                    )
                    nc.scalar.dma_start(
                        out=wkv_bf[:, 4:8, :], in_=wkv_v[:, 4:8, :]
                    )
                else:
                    nc.scalar.dma_start(
                        out=wkv_bf[:, i0 : i0 + 8, :],
                        in_=wkv_v[:, i0 : i0 + 8, :],
                    )
                nc.scalar.dma_start(
                    out=wq_bf[:, i0 : i0 + 4, :], in_=wq_v[:, i0 : i0 + 4, :]
                )
                nc.scalar.dma_start(
                    out=wq_bf[:, i0 + 4 : i0 + 8, :],
                    in_=wq_v[:, i0 + 4 : i0 + 8, :],
                )
            load_weight(wo_bf, wo_v)

            qTs = []
            anchors = []
            oths = {}
            for j in range(NCHUNK):
                qT = qtp.tile([HD, NH, CHUNK], BF16, tag="qT", name=f"qT{j}")
                qTs.append(qT)
                anchors.append(kvq_chunk(j, qT))
                if j >= 2:
                    # oth(j-2) loads, pinned after chunk j's last q-rope so
                    # their AG-done waits are satisfied when the gpsimd
                    # queue reaches them
                    oths[j - 2] = (
                        oth_load(j - 2, "a", anchors[j])
                        + oth_load(j - 2, "b", anchors[j])
                    )
                    a_mid, a_last = attention(
                        j, qT, oproj_quanta(j - 2, oths[j - 2]),
                        skip_pair0_fill=(j == 3),
                    )
                else:
                    a_mid, a_last = attention(j, qT, [])

            # tail: oproj(2) covers AG(3a/3b) wire time, then oproj(3)
            # in two head-pair phases as its AGs land.  The pin anchor is
            # attention(3)'s last ao write, so these loads sit at the very
            # end of the gpsimd stream where their waits block nothing.
            oth2 = oth_load(2, "a", a_mid) + oth_load(2, "b", a_mid)
            oth3a = oth_load(3, "a", a_last)
            oth3b = oth_load(
                3, "b", a_last, engines=(nc.gpsimd, nc.sync)
            )
            for q in oproj_quanta(2, oth2):
                q()
            oproj_tail_split(3, oth3a, oth3b)

    nc.finalize()
    return nc


def _get_graph():
    if "nc" not in _CACHED:
        _CACHED["nc"] = _build_graph()
    return _CACHED["nc"]


def _rope_tables(position_ids):
    pos = np.asarray(position_ids).reshape(-1).astype(np.float64)  # [S]
    inv_freq = 1.0 / (10000.0 ** (np.arange(0, HD, 2, dtype=np.float64) / HD))
    freqs = pos[:, None] * inv_freq[None, :]  # [S, 64]
    emb = np.concatenate([freqs, freqs], axis=-1)  # [S, HD]
    cos_t = np.cos(emb).T.astype(np.float32)  # [HD, S]
    sin_t = np.sin(emb).T.astype(np.float32)
    sin_signed = sin_t.copy()
    sin_signed[: HD // 2] *= -1.0
    bf = ml_dtypes.bfloat16
    return (
        np.ascontiguousarray(cos_t.astype(bf)),
        np.ascontiguousarray(sin_signed.astype(bf)),
    )


def kernel(hidden_states, wq, wk, wv, wo, position_ids, _trace=False):
    bf = ml_dtypes.bfloat16
    hs = np.asarray(hidden_states, np.float32).reshape(S, D)
    hsT = np.ascontiguousarray(hs.T.astype(bf))
    wq = np.asarray(wq, np.float32).astype(bf)
    wk = np.asarray(wk, np.float32).astype(bf)
    wv = np.asarray(wv, np.float32).astype(bf)
    wo = np.asarray(wo, np.float32).astype(bf)
    cos_t, sin_t = _rope_tables(position_ids)

    in_maps = []
    for c in range(NCORES):
        wkv_c = np.concatenate(
            [wk[:, HD * c : HD * (c + 1)], wv[:, HD * c : HD * (c + 1)]],
            axis=1,
        )
        in_maps.append(
            {
                "hsT": hsT,
                "wq": np.ascontiguousarray(wq[:, QCOLS * c : QCOLS * (c + 1)]),
                "wkv": np.ascontiguousarray(wkv_c),
                "wo": np.ascontiguousarray(wo[:, QCOLS * c : QCOLS * (c + 1)]),
                "cos": cos_t,
                "sin": sin_t,
            }
        )

    nc = _get_graph()
    res = run_bass_kernel_spmd(
        nc, in_maps, core_ids=list(range(NCORES)), trace=_trace
    )
    outs = [np.asarray(res.results[c]["out"]) for c in range(NCORES)]
    full = np.concatenate(outs, axis=1).reshape(1, S, D).astype(np.float32)
    if _trace:
        kernel.last_results = res
    return full
